# revision 20
# baseline (speedup 1.0000x reference)
"""Trainium2 Bass kernel for nn_Attention_56530359550323.

Full-input contract: kernel(**inputs) takes the unsharded inputs and returns
the full [4, 2048, 4096] float32 output.

Sharding: 8 cores = 4 batches (data-parallel) x 2 head-groups
(tensor-parallel over the 4 query heads; the single kv head is replicated).
Each core computes a partial output-projection [4096, 2048] (transposed);
the host sums the two partials per batch ("all-reduce after wo") and
transposes back.

Device algorithm (feature-major / transposed so every matmul has a wide
moving operand at full PE rate), structured for PE-stream density (TRN2's
PE p-state ramp punishes any gap with ~3us at half clock):

  phase 1: qT/kT/vT = W^T @ xT accumulated over 32 d-chunks. The rope +
           qk-norm + scale epilogue of s-block N is software-pipelined into
           s-block N+1's projection stream: DVE/ACT chain parts are emitted
           at dg-granular slots so the few PE ops (sum-of-squares matmul,
           reciprocal-broadcast matmul) never stall the in-order PE queue.
           DMAs are priority-ordered on the sync queue (first-needed
           weights + x tiles first, cos/sin after s-block 0's x, wo
           mid-stream) so the first matmul starts early.
  phase 2: per (q-block 512, head): scoresT = kT_chunk^T @ qT, exp on ACT,
           causal handling via diagonal-first chunk order with column
           narrowing (diagonal chunk j only computes q-columns >= j*128)
           and a single [128,128] triangular mask; PV/rowsum accumulate the
           narrowed ranges. The last s-block's epilogue is emitted AFTER
           q-block 0's attention so the PE has filler work while DVE/ACT
           run the epilogue chain.
  phase 3: partial out-projection outT[cc] = sum_h wo[h,cc]^T @ attnT_h,
           float16 partials summed on host. The last q-block emits per-head
           partials (summed on host) to shorten the dependency tail.
"""

import os
import sys
from contextlib import ExitStack

import numpy as np

if "/opt/trn_rl_repo" not in sys.path:
    sys.path.insert(0, "/opt/trn_rl_repo")

import concourse.bass as bass
import concourse.mybir as mybir
import concourse.tile as tile
from concourse import bacc, bass_utils

# ---- problem constants (hardcoded per contract) ----
B, S, D = 4, 2048, 4096
HEAD_DIM = 128
N_HEADS = 4            # local q heads in the reference module
N_KV = 1
ROPE_THETA = 500000.0
EPS = 1e-6
FLOOR_SCALE = 8192.0
ATTN_SCALE = 0.1

P = 128                # partitions
SB = 512               # s-block (q-block) size
NSB = S // SB          # 4
ND = D // P            # 32 contraction chunks for projections
NKCH = S // P          # 16 kv chunks
NCC = D // P           # 32 output column chunks
HG = 2                 # heads per group (tensor-parallel degree 2)

f32 = mybir.dt.float32
f16 = mybir.dt.float16
u8 = mybir.dt.uint8

MM_MODE = "f16"  # matmul input dtype (fp16: full PE rate, fp32 PSUM accum)

_BUILD_CACHE = {}


def build_bass():
    key = "v2"
    if key in _BUILD_CACHE:
        return _BUILD_CACHE[key]

    wdt = f16
    tdt = f16

    nc = bacc.Bacc("TRN2", target_bir_lowering=False, debug=False)

    # all big tensors arrive pre-tiled host-side so every DMA is a
    # contiguous per-partition read (avoids the 256B-1KB descriptor storm)
    xT_d = nc.dram_tensor("xT", (NSB, 8, P, 4, SB), wdt, kind="ExternalInput").ap()
    wq_d = nc.dram_tensor("wq_g", (8, P, 4, HG * HEAD_DIM), wdt, kind="ExternalInput").ap()
    wk_d = nc.dram_tensor("wk", (8, P, 4, HEAD_DIM), wdt, kind="ExternalInput").ap()
    wv_d = nc.dram_tensor("wv", (8, P, 4, HEAD_DIM), wdt, kind="ExternalInput").ap()
    wo_d = nc.dram_tensor("wo_g", (P, HG, NCC, P), wdt, kind="ExternalInput").ap()
    cs_d = nc.dram_tensor("csT", (64, S), f16, kind="ExternalInput").ap()
    sn_d = nc.dram_tensor("snT", (64, S), f16, kind="ExternalInput").ap()
    qs_d = nc.dram_tensor("qscale", (1, S), f32, kind="ExternalInput").ap()
    # qb slots 0..2 = full per-qb partials; slots 3,4 = per-head partials of
    # qb 3 (host sums them) so the device tail is one head's out-projection.
    out_d = nc.dram_tensor("outT", (NCC // 4, NSB + 1, P, 4, SB), f16, kind="ExternalOutput").ap()
    DEBUG = os.environ.get("KERNEL_DEBUG") == "1"
    if DEBUG:
        dbg_q = nc.dram_tensor("dbg_qT", (P, HG, S), tdt, kind="ExternalOutput").ap()
        dbg_k = nc.dram_tensor("dbg_kT", (P, S), tdt, kind="ExternalOutput").ap()
        dbg_v = nc.dram_tensor("dbg_v", (P, NKCH, P), tdt, kind="ExternalOutput").ap()
        dbg_a = nc.dram_tensor("dbg_aT", (P, HG, S), tdt, kind="ExternalOutput").ap()

    # full prefix+triangle masks for the 4 diagonal chunks of q-block 0
    # (full-width path): 1 => future (kill)
    masks_np = np.zeros((P, 4, SB), np.uint8)
    for c in range(4):
        kp = c * P + np.arange(P)[:, None]
        qf = np.arange(SB)[None, :]
        masks_np[:, c, :] = (kp > qf).astype(np.uint8)
    masks_d = nc.inline_tensor(masks_np, name="cmasks")
    # single triangular mask for narrowed diagonal chunks: within the
    # [128,128] diagonal block, kill local kv p > local q c
    tri_np = (np.arange(P)[:, None] > np.arange(P)[None, :]).astype(np.uint8)
    tri_d = nc.inline_tensor(tri_np, name="trimask")
    ident_d = nc.inline_tensor(np.eye(P, dtype=np.float16), name="ident")

    Exp = mybir.ActivationFunctionType.Exp
    Sqrt = mybir.ActivationFunctionType.Sqrt
    Square = mybir.ActivationFunctionType.Square

    with tile.TileContext(nc) as tc, ExitStack() as top:
        cpool = top.enter_context(tc.tile_pool(name="consts", bufs=1))
        qkpool = top.enter_context(tc.tile_pool(name="qkv", bufs=1))
        atpool = top.enter_context(tc.tile_pool(name="attn", bufs=1))

        masks_t = cpool.tile([P, 4, SB], u8)
        nc.gpsimd.dma_start(masks_t, masks_d.ap())
        tri_t = cpool.tile([P, P], u8)
        nc.gpsimd.dma_start(tri_t, tri_d.ap())
        ident_t = cpool.tile([P, P], tdt)
        nc.gpsimd.dma_start(ident_t, ident_d.ap())
        onesrow_t = cpool.tile([1, P], tdt)
        nc.vector.memset(onesrow_t, 1.0)
        onescol_t = cpool.tile([P, 1], tdt)
        nc.vector.memset(onescol_t, 1.0)
        zero_t = cpool.tile([P, SB], tdt)
        nc.vector.memset(zero_t, 0.0)
        epsb_t = cpool.tile([1, 1], f32)
        nc.vector.memset(epsb_t, float(EPS))

        # cross-phase SBUF handoff tiles
        qT_t = qkpool.tile([P, HG, S], tdt)       # normed+roped+scaled qT
        kT_t = qkpool.tile([P, S], tdt)           # normed+roped kT
        vnat_t = qkpool.tile([P, NKCH, P], tdt)   # v in natural [s, hd] tiles
        attnT_t = atpool.tile([P, HG, S], tdt)
        wo_t = qkpool.tile([P, HG, NCC, P], wdt)  # DMA'd mid phase-1
        cs_t = qkpool.tile([P, S], f16)           # rope cos (duplicated halves)
        sn_t = qkpool.tile([P, S], f16)
        qs_t = qkpool.tile([1, S], f32)

        # ---------------- phase 1: projections ----------------
        ph1w = ExitStack()   # weight/x SBUF pools (closed after the sb loop)
        ph1p = ExitStack()   # PSUM pools (closed right after the sb loop)
        epool = top.enter_context(tc.tile_pool(name="ep1", bufs=2))
        tpool = top.enter_context(tc.tile_pool(name="ropetmp", bufs=2))
        wpool = ph1w.enter_context(tc.tile_pool(name="projw", bufs=1))
        xpool = ph1w.enter_context(tc.tile_pool(name="xstream", bufs=4))
        p1ps = ph1p.enter_context(tc.tile_pool(name="p1ps", bufs=1, space="PSUM"))
        p1ps2 = ph1p.enter_context(tc.tile_pool(name="p1ps2", bufs=1, space="PSUM"))

        wq_t = wpool.tile([P, ND, HG * HEAD_DIM], wdt)
        wk_t = wpool.tile([P, ND, HEAD_DIM], wdt)
        wv_t = wpool.tile([P, ND, HEAD_DIM], wdt)

        def load_w(dg, split=False):
            dsl = slice(dg * 4, (dg + 1) * 4)
            if split:
                # head: spread the first-needed loads over four trigger
                # queues so descriptor generation runs in parallel
                nc.sync.dma_start(wq_t[:, dsl, :], wq_d[dg])
                nc.scalar.dma_start(wk_t[:, dsl, :], wk_d[dg])
                nc.scalar.dma_start(wv_t[:, dsl, :], wv_d[dg])
                return
            nc.sync.dma_start(wq_t[:, dsl, :], wq_d[dg])
            nc.sync.dma_start(wk_t[:, dsl, :], wk_d[dg])
            nc.sync.dma_start(wv_t[:, dsl, :], wv_d[dg])

        # epilogue state per (s-block, stream): (rope tile, sqrt tile)
        epi = {}

        def epi_a(sb, which, srcc, pool2, ps_full=False, with_sqrt=True):
            """Front half of rope+norm for one stream of s-block sb:
            rope (DVE), square (ACT), sum-of-squares matmul (PE),
            sqrt (ACT). srcc: [128,512] f32 copy of the projection
            accumulator (rotate-half layout)."""
            ss = slice(sb * SB, (sb + 1) * SB)
            rope = epool.tile([P, SB], f32, tag=f"rope{which}", name=f"rope{which}")
            ta = tpool.tile([64, SB], f32, tag="ta")
            tb = tpool.tile([64, SB], f32, tag="tb")
            te, to = srcc[0:64, :], srcc[64:128, :]
            nc.vector.tensor_mul(ta, te, cs_t[0:64, ss])
            nc.vector.tensor_mul(tb, to, sn_t[64:128, ss])
            nc.vector.tensor_sub(rope[0:64, :], ta, tb)
            tc_ = tpool.tile([64, SB], f32, tag="tc")
            td = tpool.tile([64, SB], f32, tag="td")
            nc.vector.tensor_mul(tc_, to, cs_t[64:128, ss])
            nc.vector.tensor_mul(td, te, sn_t[0:64, ss])
            nc.vector.tensor_add(rope[64:128, :], tc_, td)

            sq = epool.tile([P, SB], tdt, tag="sq")
            nc.scalar.activation(sq, rope, Square)
            if ps_full:
                ss_ps = pool2.tile([P, SB], f32, tag="ps", bufs=4, name="ssps")[0:1, :]
            else:
                ss_ps = pool2.tile([1, SB], f32, tag="ss", bufs=2, name="ssps")
            nc.tensor.matmul(
                ss_ps, onescol_t[:], sq[:], start=True, stop=True
            )
            if with_sqrt:
                sqr = epool.tile([1, SB], f32, tag=f"sqr{which}", name=f"sqr{which}")
                nc.scalar.activation(
                    sqr, ss_ps, Sqrt, bias=epsb_t[:], scale=1.0 / HEAD_DIM)
                epi[(sb, which)] = (rope, sqr)
            else:
                # stage the sum of squares to SBUF (Copy: no table switch)
                # so the PSUM slot can rotate before the deferred Sqrt runs
                ssb = epool.tile([1, SB], f32, tag=f"ssb{which}", name=f"ssb{which}")
                nc.scalar.copy(ssb, ss_ps)
                epi[(sb, which)] = (rope, ssb)

        def epi_sqrt(sb, which):
            rope, ssb = epi.pop((sb, which))
            sqr = epool.tile([1, SB], f32, tag=f"sqr{which}", name=f"sqr{which}")
            nc.scalar.activation(
                sqr, ssb, Sqrt, bias=epsb_t[:], scale=1.0 / HEAD_DIM)
            epi[(sb, which)] = (rope, sqr)

        def epi_b(sb, which, dst, with_qscale, pool2, ps_full=False):
            """Back half: qscale (DVE), broadcast matmul (PE),
            PSUM->SBUF stage (ACT), final normalize multiply (DVE)."""
            ss = slice(sb * SB, (sb + 1) * SB)
            rope, sqr = epi.pop((sb, which))
            rec = epool.tile([1, SB], f32, tag="recf")
            nc.vector.reciprocal_approx_fast(out=rec, in_=sqr)
            if with_qscale:
                rec2 = epool.tile([1, SB], f32, tag="rec2")
                nc.vector.tensor_mul(rec2, rec, qs_t[:, ss])
                rec = rec2
            recr = epool.tile([1, SB], tdt, tag="recr")
            nc.vector.tensor_copy(recr, rec)
            if ps_full:
                bc_ps = pool2.tile([P, SB], f32, tag="ps", bufs=4, name="bcps")
            else:
                bc_ps = pool2.tile([P, SB], f32, tag="bc", bufs=1, name="bcps")
            nc.tensor.matmul(
                bc_ps, onesrow_t[:], recr[:], start=True, stop=True
            )
            # DVE tensor-tensor with a PSUM operand runs ~6x slow; stage
            # the broadcast through ACT (line-rate PSUM reads) instead.
            bcs = epool.tile([P, SB], f32, tag="bcs")
            nc.scalar.copy(bcs, bc_ps)
            nc.vector.tensor_mul(dst, rope, bcs)

        def vtrans(sb, v_sb):
            for t in range(4):
                tp_ps = p1ps2.tile([P, P], tdt, tag="tp", bufs=1, name="tpps")
                nc.tensor.transpose(tp_ps, v_sb[:, t * P:(t + 1) * P], ident_t[:])
                nc.vector.tensor_copy(vnat_t[:, sb * 4 + t, :], tp_ps)

        prev = None  # (sb, copies, kc, v_sb) of previous s-block
        for sb in range(NSB):
            q_ps = [
                p1ps.tile([P, SB], f32, tag=f"q{h}", name=f"qps{h}")
                for h in range(HG)
            ]
            k_ps = p1ps.tile([P, SB], f32, tag="k")
            v_ps = p1ps.tile([P, SB], f32, tag="v")
            for dg in range(8):
                if sb == 0:
                    load_w(dg, split=(dg == 0))
                xt = xpool.tile([P, 4, SB], wdt)
                if sb == 0 and dg == 0:
                    nc.scalar.dma_start(xt, xT_d[sb, dg])
                elif sb == 0 and dg == 1:
                    nc.gpsimd.dma_start(xt, xT_d[sb, dg])
                else:
                    nc.sync.dma_start(xt, xT_d[sb, dg])
                if sb == 1 and dg == 0:
                    # rope tables + qscale: needed from s-block 0's epilogue
                    # (during P(1)); same queue, behind the first x tiles.
                    nc.sync.dma_start(cs_t[0:64, :], cs_d)
                    nc.sync.dma_start(cs_t[64:128, :], cs_d)
                    nc.sync.dma_start(sn_t[0:64, :], sn_d)
                    nc.sync.dma_start(sn_t[64:128, :], sn_d)
                    nc.sync.dma_start(qs_t, qs_d)
                if sb == 2 and dg == 4:
                    nc.sync.dma_start(wo_t, wo_d)
                for c in range(4):
                    d = dg * 4 + c
                    st, sp = (d == 0), (d == ND - 1)
                    rhs = xt[:, c, :]
                    for h in range(HG):
                        nc.tensor.matmul(
                            q_ps[h],
                            wq_t[:, d, h * P:(h + 1) * P],
                            rhs,
                            start=st,
                            stop=sp,
                        )
                    nc.tensor.matmul(k_ps, wk_t[:, d, :], rhs, start=st, stop=sp)
                    nc.tensor.matmul(v_ps, wv_t[:, d, :], rhs, start=st, stop=sp)
                # software-pipelined epilogue of the previous s-block,
                # emitted at dg slots so the PE never waits on DVE/ACT
                if prev is not None:
                    psb, pcopies, pkc, pv_sb = prev
                    pss = slice(psb * SB, (psb + 1) * SB)
                    if dg == 3:
                        epi_a(psb, "q0", pcopies[0], p1ps2)
                    elif dg == 4:
                        epi_a(psb, "q1", pcopies[1], p1ps2)
                    elif dg == 5:
                        epi_a(psb, "k", pkc, p1ps2)
                    elif dg == 6:
                        epi_b(psb, "q0", qT_t[:, 0, pss], True, p1ps2)
                    elif dg == 7:
                        epi_b(psb, "q1", qT_t[:, 1, pss], True, p1ps2)
                        epi_b(psb, "k", kT_t[:, pss], False, p1ps2)
                        vtrans(psb, pv_sb)

            # free the accumulator banks quickly; on the last s-block the
            # v copy goes first so its transposes unblock immediately
            v_sb = None
            if sb == NSB - 1:
                v_sb = epool.tile([P, SB], tdt, tag="vc", bufs=2)
                nc.vector.tensor_copy(v_sb, v_ps)
            copies = []
            for h in range(HG):
                qc = epool.tile([P, SB], f32, tag=f"qc{h}", bufs=2,
                                name=f"qcopy{h}")
                nc.vector.tensor_copy(qc, q_ps[h])
                copies.append(qc)
            kc = epool.tile([P, SB], f32, tag="kc", bufs=2)
            nc.vector.tensor_copy(kc, k_ps)
            if v_sb is None:
                v_sb = epool.tile([P, SB], tdt, tag="vc", bufs=2)
                nc.vector.tensor_copy(v_sb, v_ps)
            prev = (sb, copies, kc, v_sb)

        # last s-block's v transposes only depend on the v copy: emit now,
        # while phase-1 PSUM banks are still open.
        lsb, lcopies, lkc, lv_sb = prev
        vtrans(lsb, lv_sb)
        lss = slice(lsb * SB, (lsb + 1) * SB)
        ph1p.close()
        ph1w.close()

        # ---------------- phase 2: attention ----------------
        e2pool = top.enter_context(tc.tile_pool(name="ep2", bufs=2))
        o3pool = top.enter_context(tc.tile_pool(name="oc", bufs=8))
        expool = top.enter_context(tc.tile_pool(name="exps", bufs=12))
        p2ps = top.enter_context(tc.tile_pool(name="p2ps", bufs=2, space="PSUM"))
        p2sc = top.enter_context(tc.tile_pool(name="p2sc", bufs=1, space="PSUM"))

        def finalize(h, qs_sl, pv_ps, rs_ps):
            # normalization chain, emitted one group late so its PE
            # broadcast matmul never stalls the in-order PE stream.
            pvs = e2pool.tile([P, SB], f32, tag="pvs")
            nc.scalar.copy(pvs, pv_ps)
            rec = e2pool.tile([1, SB], f32, tag="rec")
            nc.vector.reciprocal_approx_fast(out=rec, in_=rs_ps)
            recr = e2pool.tile([1, SB], tdt, tag="recr")
            nc.vector.tensor_copy(recr, rec)
            bc_ps = p2sc.tile([P, SB], f32, tag="ps", bufs=4, name="obc")
            nc.tensor.matmul(
                bc_ps, onesrow_t[:], recr[:], start=True, stop=True,
            )
            bc_sb = e2pool.tile([P, SB], f32, tag="bcc")
            nc.vector.tensor_copy(bc_sb, bc_ps)
            nc.vector.tensor_mul(attnT_t[:, h, qs_sl], pvs, bc_sb)

        def outproj(qb, heads, slot, ccr=None):
            # output written 4 column-chunks per DMA (4KB/partition packets
            # instead of 1KB, and 4x fewer descriptor-generation triggers)
            qsl = slice(qb * SB, (qb + 1) * SB)
            o4 = None
            for cc in (ccr if ccr is not None else range(NCC)):
                o_ps = p2sc.tile([P, SB], f32, tag="ps", bufs=4, name="ops")
                for i, h in enumerate(heads):
                    nc.tensor.matmul(
                        o_ps,
                        wo_t[:, h, cc, :],
                        attnT_t[:, h, qsl],
                        start=(i == 0),
                        stop=(i == len(heads) - 1),
                    )
                if cc % 4 == 0:
                    o4 = o3pool.tile([P, 4, SB], f16, tag="oc", bufs=3)
                # ACT is the busy engine in late phase 2 (exps); bias the
                # PSUM->SBUF staging copies 3:1 toward DVE
                if cc % 4 == 1:
                    nc.scalar.copy(o4[:, cc % 4, :], o_ps)
                else:
                    nc.vector.tensor_copy(o4[:, cc % 4, :], o_ps)
                if cc % 4 == 3:
                    nc.gpsimd.dma_start(out_d[cc // 4, slot], o4)

        def attn_group(qb, h, interject=None):
            """Emit one (q-block, head) attention group with 2-deep score
            prefetch. Diagonal-first chunk order with column narrowing; the
            final chunk is always full-width so PSUM accumulation groups
            start/stop over the whole bank. interject: {chunk_idx: fn}
            emits extra (independent) work after that chunk's matmuls."""
            qs_sl = slice(qb * SB, (qb + 1) * SB)
            pv_ps = p2ps.tile([P, SB], f32, tag="pv")
            rs_ps = p2ps.tile([1, SB], f32, tag="rs")
            qt = qT_t[:, h, qs_sl]
            # chunk: (kv_chunk, col0 for sc/exp, pv col0, mask)
            # mask: ("old", j) full-width prefix+triangle | ("tri",) 128-wide
            if qb == 0:
                chunks = [(j, j * P, j * P, ("tri",)) for j in range(3)]
                chunks += [(3, 3 * P, 0, ("old", 3))]
            else:
                chunks = [(4 * qb + j, j * P, j * P, ("tri",)) for j in range(4)]
                chunks += [(c, 0, 0, None) for c in range(4 * qb)]
            n = len(chunks)

            sc_tiles = [None] * n

            def emit_sc(i):
                c, col0, _, _ = chunks[i]
                sc_ps = p2sc.tile([P, SB], f32, tag="ps", bufs=4, name="scps")
                nc.tensor.matmul(
                    sc_ps[:, col0:],
                    kT_t[:, c * P:(c + 1) * P],
                    qt[:, col0:],
                    start=True,
                    stop=True,
                )
                sc_tiles[i] = sc_ps

            def emit_rest(i):
                c, col0, pcol0, mask = chunks[i]
                sc_ps = sc_tiles[i]
                sc_tiles[i] = None
                e_sb = expool.tile([P, SB], tdt, tag="exp")
                nc.scalar.activation(e_sb[:, col0:], sc_ps[:, col0:], Exp)
                if mask is not None:
                    if mask[0] == "old":
                        # prefix+triangle mask also zeroes the (uncomputed)
                        # e_sb columns below col0, enabling full-width pv/rs
                        nc.vector.copy_predicated(
                            e_sb, masks_t[:, mask[1], :], zero_t
                        )
                    else:
                        nc.vector.copy_predicated(
                            e_sb[:, col0:col0 + P], tri_t, zero_t[:, 0:P]
                        )
                st, sp = (i == 0), (i == n - 1)
                nc.tensor.matmul(
                    pv_ps[:, pcol0:], vnat_t[:, c, :], e_sb[:, pcol0:],
                    start=st, stop=sp,
                )
                nc.tensor.matmul(
                    rs_ps[:, pcol0:], onescol_t[:], e_sb[:, pcol0:],
                    start=st, stop=sp,
                )
                if interject and i in interject:
                    interject[i]()

            for i in range(n):
                emit_sc(i)
                if i >= 2:
                    emit_rest(i - 2)
            emit_rest(n - 2)
            emit_rest(n - 1)
            return (qb, h, qs_sl, pv_ps, rs_ps)

        # The last s-block's rope/norm epilogue is interleaved into early
        # phase 2: its outputs are only needed by q-block 3's attention,
        # and the q-block 0/1 groups + finalizes give the PE filler work
        # while the epilogue's DVE/ACT chains drain. All ACT functions
        # (Exp/Ln/Square/Copy) live in one table, so interleaving is free.
        g00 = attn_group(0, 0)
        epi_a(lsb, "q0", lcopies[0], p2sc, ps_full=True, with_sqrt=False)
        g01 = attn_group(0, 1)
        epi_a(lsb, "q1", lcopies[1], p2sc, ps_full=True, with_sqrt=False)

        if DEBUG:
            nc.sync.dma_start(dbg_q, qT_t)
            nc.sync.dma_start(dbg_k, kT_t)
            nc.sync.dma_start(dbg_v, vnat_t)

        # remaining groups, finalize one group late; outproj when both
        # heads of a q-block are done. Last q-block: per-head partials.
        finalize(g00[1], *g00[2:])
        g10 = attn_group(1, 0, interject={
            6: lambda: epi_a(lsb, "k", lkc, p2sc, ps_full=True,
                             with_sqrt=False)})
        finalize(g01[1], *g01[2:])
        outproj(0, [0, 1], 0)
        # three consecutive Sqrts: exactly one table switch out of Exp's
        # table and one back, with outproj/attention mms as PE filler
        epi_sqrt(lsb, "q0")
        epi_sqrt(lsb, "q1")
        epi_sqrt(lsb, "k")
        epi_b(lsb, "q0", qT_t[:, 0, lss], True, p2sc, ps_full=True)
        epi_b(lsb, "q1", qT_t[:, 1, lss], True, p2sc, ps_full=True)
        epi_b(lsb, "k", kT_t[:, lss], False, p2sc, ps_full=True)
        g11 = attn_group(1, 1)
        finalize(g10[1], *g10[2:])
        g20 = attn_group(2, 0)
        finalize(g11[1], *g11[2:])
        outproj(1, [0, 1], 1)
        g21 = attn_group(2, 1)
        finalize(g20[1], *g20[2:])
        g30 = attn_group(3, 0)
        finalize(g21[1], *g21[2:])
        outproj(2, [0, 1], 2)
        # tail: the finalize chains and h0's projection interleave into the
        # last attention group so only h1's projection trails the stream.
        g31 = attn_group(3, 1, interject={
            5: lambda: finalize(g30[1], *g30[2:]),
            10: lambda: outproj(3, [0], 3, range(0, 16))})
        finalize(g31[1], *g31[2:])
        outproj(3, [0], 3, range(16, NCC))
        outproj(3, [1], 4)

        if DEBUG:
            nc.sync.dma_start(dbg_a, attnT_t)

    nc.compile()
    _BUILD_CACHE[key] = nc
    return nc


def _host_prep(x, positions, wq, wk, wv, wo):
    """Returns per-core input maps."""
    npdt = np.float16

    pos_f = positions.astype(np.float32)
    inv_freq = (
        1.0
        / (ROPE_THETA ** (np.arange(0, HEAD_DIM, 2, dtype=np.float32) / HEAD_DIM))
    ).astype(np.float32)
    ang = pos_f[:, None] * inv_freq[None, :]        # [S, 64]
    csT = np.ascontiguousarray(np.cos(ang).T.astype(np.float16))  # [64, S]
    snT = np.ascontiguousarray(np.sin(ang).T.astype(np.float16))  # [64, S]
    attn_scales = (
        np.log(np.floor((pos_f + 1.0) / FLOOR_SCALE) + 1.0) * ATTN_SCALE + 1.0
    )
    qscale = (attn_scales / np.sqrt(np.float32(HEAD_DIM))).astype(np.float32)[None, :]

    # rotate-half permutation of q/k feature dims (per head), folded into
    # the projection weight columns: permuted feature j<64 <- 2j, j>=64 <- 2(j-64)+1
    perm = np.concatenate([np.arange(0, HEAD_DIM, 2), np.arange(1, HEAD_DIM, 2)])
    wq_p = wq.reshape(D, N_HEADS, HEAD_DIM)[:, :, perm].reshape(D, N_HEADS * HEAD_DIM)
    wk_p = wk[:, perm]

    def tile_x(xT):
        # [D, S] -> [sb, dg, p, c, s]
        return np.ascontiguousarray(
            xT.reshape(8, 4, P, NSB, SB).transpose(3, 0, 2, 1, 4)
        )

    def tile_w(w):
        # [D, m] -> [dg, p, c, m]
        m = w.shape[1]
        return np.ascontiguousarray(
            w.reshape(8, 4, P, m).transpose(0, 2, 1, 3)
        )

    def tile_wo(wg):
        # [256, D] -> [p, hh, cc, q]
        return np.ascontiguousarray(
            wg.reshape(HG, P, NCC, P).transpose(1, 0, 2, 3)
        )

    in_maps = []
    for core in range(8):
        b, g = core // 2, core % 2
        xT = np.ascontiguousarray(x[b].T).astype(npdt, copy=False)
        in_maps.append(
            {
                "xT": tile_x(xT),
                "wq_g": tile_w(
                    wq_p[:, g * HG * HEAD_DIM:(g + 1) * HG * HEAD_DIM].astype(npdt)
                ),
                "wk": tile_w(wk_p.astype(npdt)),
                "wv": tile_w(wv.astype(npdt)),
                "wo_g": tile_wo(
                    wo[g * HG * HEAD_DIM:(g + 1) * HG * HEAD_DIM, :].astype(npdt)
                ),
                "csT": csT,
                "snT": snT,
                "qscale": qscale,
            }
        )
    return in_maps


def kernel(x, positions, wq, wk, wv, wo, _trace=False, _trace_kwargs=None):
    x = np.asarray(x, np.float32)
    positions = np.asarray(positions)
    wq = np.asarray(wq, np.float32)
    wk = np.asarray(wk, np.float32)
    wv = np.asarray(wv, np.float32)
    wo = np.asarray(wo, np.float32)

    nc = build_bass()
    in_maps = _host_prep(x, positions, wq, wk, wv, wo)
    res = bass_utils.run_bass_kernel_spmd(
        nc, in_maps, core_ids=list(range(8)), trace=_trace,
        **(_trace_kwargs or {}),
    )
    kernel.last_results = res

    out = np.empty((B, S, D), np.float32)
    for b in range(B):
        pa = res.results[2 * b]["outT"].astype(np.float32)
        pb = res.results[2 * b + 1]["outT"].astype(np.float32)
        # [cc4, slot, p, c, s]
        pa[:, 3] += pa[:, 4]
        pb[:, 3] += pb[:, 4]
        comb = pa[:, :NSB] + pb[:, :NSB]          # [cc4, qb, p, c, s]
        full = comb.transpose(0, 3, 2, 1, 4).reshape(D, S)
        out[b] = full.T
    return out


# revision 22
# speedup vs baseline: 1.0440x; 1.0440x over previous
"""Trainium2 Bass kernel for nn_Attention_56530359550323.

Full-input contract: kernel(**inputs) takes the unsharded inputs and returns
the full [4, 2048, 4096] float32 output.

Sharding: 8 cores = 4 batches (data-parallel) x 2 head-groups
(tensor-parallel over the 4 query heads; the single kv head is replicated).
Each core computes a partial output-projection [4096, 2048] (transposed);
the host sums the two partials per batch ("all-reduce after wo") and
transposes back.

Device algorithm (feature-major / transposed so every matmul has a wide
moving operand at full PE rate), structured for PE-stream density (TRN2's
PE p-state ramp punishes any gap with ~3us at half clock):

  phase 1: qT/kT/vT = W^T @ xT accumulated over 32 d-chunks. The rope +
           qk-norm + scale epilogue of s-block N is software-pipelined into
           s-block N+1's projection stream: DVE/ACT chain parts are emitted
           at dg-granular slots so the few PE ops (sum-of-squares matmul,
           reciprocal-broadcast matmul) never stall the in-order PE queue.
           DMAs are priority-ordered on the sync queue (first-needed
           weights + x tiles first, cos/sin after s-block 0's x, wo
           mid-stream) so the first matmul starts early.
  phase 2: per (q-block 512, head): scoresT = kT_chunk^T @ qT, exp on ACT,
           causal handling via diagonal-first chunk order with column
           narrowing (diagonal chunk j only computes q-columns >= j*128)
           and a single [128,128] triangular mask; PV/rowsum accumulate the
           narrowed ranges. The last s-block's epilogue is emitted AFTER
           q-block 0's attention so the PE has filler work while DVE/ACT
           run the epilogue chain.
  phase 3: partial out-projection outT[cc] = sum_h wo[h,cc]^T @ attnT_h,
           float16 partials summed on host. The last q-block emits per-head
           partials (summed on host) to shorten the dependency tail.
"""

import os
import sys
from contextlib import ExitStack

import numpy as np

if "/opt/trn_rl_repo" not in sys.path:
    sys.path.insert(0, "/opt/trn_rl_repo")

import concourse.bass as bass
import concourse.mybir as mybir
import concourse.tile as tile
from concourse import bacc, bass_utils

# ---- problem constants (hardcoded per contract) ----
B, S, D = 4, 2048, 4096
HEAD_DIM = 128
N_HEADS = 4            # local q heads in the reference module
N_KV = 1
ROPE_THETA = 500000.0
EPS = 1e-6
FLOOR_SCALE = 8192.0
ATTN_SCALE = 0.1

P = 128                # partitions
SB = 512               # s-block (q-block) size
NSB = S // SB          # 4
ND = D // P            # 32 contraction chunks for projections
NKCH = S // P          # 16 kv chunks
NCC = D // P           # 32 output column chunks
HG = 2                 # heads per group (tensor-parallel degree 2)

f32 = mybir.dt.float32
f16 = mybir.dt.float16
u8 = mybir.dt.uint8

MM_MODE = "f16"  # matmul input dtype (fp16: full PE rate, fp32 PSUM accum)

_BUILD_CACHE = {}


def build_bass():
    key = "v2"
    if key in _BUILD_CACHE:
        return _BUILD_CACHE[key]

    wdt = f16
    tdt = f16

    nc = bacc.Bacc("TRN2", target_bir_lowering=False, debug=False)

    # all big tensors arrive pre-tiled host-side so every DMA is a
    # contiguous per-partition read (avoids the 256B-1KB descriptor storm)
    xT_d = nc.dram_tensor("xT", (NSB, 8, P, 4, SB), wdt, kind="ExternalInput").ap()
    wq_d = nc.dram_tensor("wq_g", (8, P, 4, HG * HEAD_DIM), wdt, kind="ExternalInput").ap()
    wk_d = nc.dram_tensor("wk", (8, P, 4, HEAD_DIM), wdt, kind="ExternalInput").ap()
    wv_d = nc.dram_tensor("wv", (8, P, 4, HEAD_DIM), wdt, kind="ExternalInput").ap()
    wo_d = nc.dram_tensor("wo_g", (P, HG, NCC, P), wdt, kind="ExternalInput").ap()
    cs_d = nc.dram_tensor("csT", (64, S), f16, kind="ExternalInput").ap()
    sn_d = nc.dram_tensor("snT", (64, S), f16, kind="ExternalInput").ap()
    qs_d = nc.dram_tensor("qscale", (1, S), f32, kind="ExternalInput").ap()
    # qb slots 0..2 = full per-qb partials; slots 3,4 = per-head partials of
    # qb 3 (host sums them) so the device tail is one head's out-projection.
    out_d = nc.dram_tensor("outT", (NCC // 4, NSB + 1, P, 4, SB), f16, kind="ExternalOutput").ap()
    DEBUG = os.environ.get("KERNEL_DEBUG") == "1"
    if DEBUG:
        dbg_q = nc.dram_tensor("dbg_qT", (P, HG, S), tdt, kind="ExternalOutput").ap()
        dbg_k = nc.dram_tensor("dbg_kT", (P, S), tdt, kind="ExternalOutput").ap()
        dbg_v = nc.dram_tensor("dbg_v", (P, NKCH, P), tdt, kind="ExternalOutput").ap()
        dbg_a = nc.dram_tensor("dbg_aT", (P, HG, S), tdt, kind="ExternalOutput").ap()

    # full prefix+triangle masks for the 4 diagonal chunks of q-block 0
    # (full-width path): 1 => future (kill)
    masks_np = np.zeros((P, 4, SB), np.uint8)
    for c in range(4):
        kp = c * P + np.arange(P)[:, None]
        qf = np.arange(SB)[None, :]
        masks_np[:, c, :] = (kp > qf).astype(np.uint8)
    masks_d = nc.inline_tensor(masks_np, name="cmasks")
    # single triangular mask for narrowed diagonal chunks: within the
    # [128,128] diagonal block, kill local kv p > local q c
    tri_np = (np.arange(P)[:, None] > np.arange(P)[None, :]).astype(np.uint8)
    tri_d = nc.inline_tensor(tri_np, name="trimask")
    ident_d = nc.inline_tensor(np.eye(P, dtype=np.float16), name="ident")

    Exp = mybir.ActivationFunctionType.Exp
    Sqrt = mybir.ActivationFunctionType.Sqrt
    Square = mybir.ActivationFunctionType.Square

    with tile.TileContext(nc) as tc, ExitStack() as top:
        cpool = top.enter_context(tc.tile_pool(name="consts", bufs=1))
        qkpool = top.enter_context(tc.tile_pool(name="qkv", bufs=1))
        atpool = top.enter_context(tc.tile_pool(name="attn", bufs=1))

        masks_t = cpool.tile([P, 4, SB], u8)
        nc.gpsimd.dma_start(masks_t, masks_d.ap())
        tri_t = cpool.tile([P, P], u8)
        nc.gpsimd.dma_start(tri_t, tri_d.ap())
        ident_t = cpool.tile([P, P], tdt)
        nc.gpsimd.dma_start(ident_t, ident_d.ap())
        onesrow_t = cpool.tile([1, P], tdt)
        nc.vector.memset(onesrow_t, 1.0)
        onescol_t = cpool.tile([P, 1], tdt)
        nc.vector.memset(onescol_t, 1.0)
        zero_t = cpool.tile([P, SB], tdt)
        nc.vector.memset(zero_t, 0.0)
        epsb_t = cpool.tile([1, 1], f32)
        nc.vector.memset(epsb_t, float(EPS))

        # cross-phase SBUF handoff tiles
        qT_t = qkpool.tile([P, HG, S], tdt)       # normed+roped+scaled qT
        kT_t = qkpool.tile([P, S], tdt)           # normed+roped kT
        vnat_t = qkpool.tile([P, NKCH, P], tdt)   # v in natural [s, hd] tiles
        attnT_t = atpool.tile([P, HG, S], tdt)
        wo_t = qkpool.tile([P, HG, NCC, P], wdt)  # DMA'd mid phase-1
        cs_t = qkpool.tile([P, S], f16)           # rope cos (duplicated halves)
        sn_t = qkpool.tile([P, S], f16)
        qs_t = qkpool.tile([1, S], f32)

        # ---------------- phase 1: projections ----------------
        ph1w = ExitStack()   # weight/x SBUF pools (closed after the sb loop)
        ph1p = ExitStack()   # PSUM pools (closed right after the sb loop)
        epool = top.enter_context(tc.tile_pool(name="ep1", bufs=2))
        tpool = top.enter_context(tc.tile_pool(name="ropetmp", bufs=2))
        wpool = ph1w.enter_context(tc.tile_pool(name="projw", bufs=1))
        xpool = ph1w.enter_context(tc.tile_pool(name="xstream", bufs=4))
        p1ps = ph1p.enter_context(tc.tile_pool(name="p1ps", bufs=1, space="PSUM"))
        p1ps2 = ph1p.enter_context(tc.tile_pool(name="p1ps2", bufs=1, space="PSUM"))

        wq_t = wpool.tile([P, ND, HG * HEAD_DIM], wdt)
        wk_t = wpool.tile([P, ND, HEAD_DIM], wdt)
        wv_t = wpool.tile([P, ND, HEAD_DIM], wdt)

        def load_w(dg, split=False):
            dsl = slice(dg * 4, (dg + 1) * 4)
            if split:
                # head: spread the first-needed loads over four trigger
                # queues so descriptor generation runs in parallel
                nc.sync.dma_start(wq_t[:, dsl, :], wq_d[dg])
                nc.scalar.dma_start(wk_t[:, dsl, :], wk_d[dg])
                nc.scalar.dma_start(wv_t[:, dsl, :], wv_d[dg])
                return
            nc.sync.dma_start(wq_t[:, dsl, :], wq_d[dg])
            nc.sync.dma_start(wk_t[:, dsl, :], wk_d[dg])
            nc.sync.dma_start(wv_t[:, dsl, :], wv_d[dg])

        # epilogue state per (s-block, stream): (rope tile, sqrt tile)
        epi = {}

        def epi_a(sb, which, srcc, pool2, ps_full=False, with_sqrt=True):
            """Front half of rope+norm for one stream of s-block sb:
            rope (DVE), square (ACT), sum-of-squares matmul (PE),
            sqrt (ACT). srcc: [128,512] f32 copy of the projection
            accumulator (rotate-half layout)."""
            ss = slice(sb * SB, (sb + 1) * SB)
            rope = epool.tile([P, SB], f32, tag=f"rope{which}", name=f"rope{which}")
            ta = tpool.tile([64, SB], f32, tag="ta")
            tb = tpool.tile([64, SB], f32, tag="tb")
            te, to = srcc[0:64, :], srcc[64:128, :]
            nc.vector.tensor_mul(ta, te, cs_t[0:64, ss])
            nc.vector.tensor_mul(tb, to, sn_t[64:128, ss])
            nc.vector.tensor_sub(rope[0:64, :], ta, tb)
            tc_ = tpool.tile([64, SB], f32, tag="tc")
            td = tpool.tile([64, SB], f32, tag="td")
            nc.vector.tensor_mul(tc_, to, cs_t[64:128, ss])
            nc.vector.tensor_mul(td, te, sn_t[0:64, ss])
            nc.vector.tensor_add(rope[64:128, :], tc_, td)

            sq = epool.tile([P, SB], tdt, tag="sq")
            nc.scalar.activation(sq, rope, Square)
            if ps_full:
                ss_ps = pool2.tile([P, SB], f32, tag="ps", bufs=4, name="ssps")[0:1, :]
            else:
                ss_ps = pool2.tile([1, SB], f32, tag="ss", bufs=2, name="ssps")
            nc.tensor.matmul(
                ss_ps, onescol_t[:], sq[:], start=True, stop=True
            )
            if with_sqrt:
                sqr = epool.tile([1, SB], f32, tag=f"sqr{which}", name=f"sqr{which}")
                nc.scalar.activation(
                    sqr, ss_ps, Sqrt, bias=epsb_t[:], scale=1.0 / HEAD_DIM)
                epi[(sb, which)] = (rope, sqr)
            else:
                # stage the sum of squares to SBUF (Copy: no table switch)
                # so the PSUM slot can rotate before the deferred Sqrt runs
                ssb = epool.tile([1, SB], f32, tag=f"ssb{which}", name=f"ssb{which}")
                nc.scalar.copy(ssb, ss_ps)
                epi[(sb, which)] = (rope, ssb)

        def epi_sqrt(sb, which):
            rope, ssb = epi.pop((sb, which))
            sqr = epool.tile([1, SB], f32, tag=f"sqr{which}", name=f"sqr{which}")
            nc.scalar.activation(
                sqr, ssb, Sqrt, bias=epsb_t[:], scale=1.0 / HEAD_DIM)
            epi[(sb, which)] = (rope, sqr)

        def epi_b(sb, which, dst, with_qscale, pool2, ps_full=False):
            """Back half: qscale (DVE), broadcast matmul (PE),
            PSUM->SBUF stage (ACT), final normalize multiply (DVE)."""
            ss = slice(sb * SB, (sb + 1) * SB)
            rope, sqr = epi.pop((sb, which))
            rec = epool.tile([1, SB], f32, tag="recf")
            nc.vector.reciprocal_approx_fast(out=rec, in_=sqr)
            if with_qscale:
                rec2 = epool.tile([1, SB], f32, tag="rec2")
                nc.vector.tensor_mul(rec2, rec, qs_t[:, ss])
                rec = rec2
            recr = epool.tile([1, SB], tdt, tag="recr")
            nc.vector.tensor_copy(recr, rec)
            if ps_full:
                bc_ps = pool2.tile([P, SB], f32, tag="ps", bufs=4, name="bcps")
            else:
                bc_ps = pool2.tile([P, SB], f32, tag="bc", bufs=1, name="bcps")
            nc.tensor.matmul(
                bc_ps, onesrow_t[:], recr[:], start=True, stop=True
            )
            # DVE tensor-tensor with a PSUM operand runs ~6x slow; stage
            # the broadcast through ACT (line-rate PSUM reads) instead.
            bcs = epool.tile([P, SB], f32, tag="bcs")
            nc.scalar.copy(bcs, bc_ps)
            nc.vector.tensor_mul(dst, rope, bcs)

        def vtrans(sb, v_sb):
            for t in range(4):
                tp_ps = p1ps2.tile([P, P], tdt, tag="tp", bufs=1, name="tpps")
                nc.tensor.transpose(tp_ps, v_sb[:, t * P:(t + 1) * P], ident_t[:])
                nc.vector.tensor_copy(vnat_t[:, sb * 4 + t, :], tp_ps)

        prev = None  # (sb, copies, kc, v_sb) of previous s-block
        for sb in range(NSB):
            q_ps = [
                p1ps.tile([P, SB], f32, tag=f"q{h}", name=f"qps{h}")
                for h in range(HG)
            ]
            k_ps = p1ps.tile([P, SB], f32, tag="k")
            v_ps = p1ps.tile([P, SB], f32, tag="v")
            for dg in range(8):
                if sb == 0:
                    load_w(dg, split=(dg == 0))
                xt = xpool.tile([P, 4, SB], wdt)
                if sb == 0 and dg == 0:
                    nc.scalar.dma_start(xt, xT_d[sb, dg])
                elif sb == 0 and dg == 1:
                    nc.gpsimd.dma_start(xt, xT_d[sb, dg])
                else:
                    nc.sync.dma_start(xt, xT_d[sb, dg])
                if sb == 1 and dg == 0:
                    # rope tables + qscale: needed from s-block 0's epilogue
                    # (during P(1)); same queue, behind the first x tiles.
                    nc.sync.dma_start(cs_t[0:64, :], cs_d)
                    nc.sync.dma_start(cs_t[64:128, :], cs_d)
                    nc.sync.dma_start(sn_t[0:64, :], sn_d)
                    nc.sync.dma_start(sn_t[64:128, :], sn_d)
                    nc.sync.dma_start(qs_t, qs_d)
                if sb == 2 and dg == 4:
                    nc.sync.dma_start(wo_t, wo_d)
                for c in range(4):
                    d = dg * 4 + c
                    st, sp = (d == 0), (d == ND - 1)
                    rhs = xt[:, c, :]
                    for h in range(HG):
                        nc.tensor.matmul(
                            q_ps[h],
                            wq_t[:, d, h * P:(h + 1) * P],
                            rhs,
                            start=st,
                            stop=sp,
                        )
                    nc.tensor.matmul(k_ps, wk_t[:, d, :], rhs, start=st, stop=sp)
                    nc.tensor.matmul(v_ps, wv_t[:, d, :], rhs, start=st, stop=sp)
                # software-pipelined epilogue of the previous s-block,
                # emitted at dg slots so the PE never waits on DVE/ACT
                if prev is not None:
                    psb, pcopies, pkc, pv_sb = prev
                    pss = slice(psb * SB, (psb + 1) * SB)
                    if dg == 3:
                        epi_a(psb, "q0", pcopies[0], p1ps2)
                    elif dg == 4:
                        epi_a(psb, "q1", pcopies[1], p1ps2)
                    elif dg == 5:
                        epi_a(psb, "k", pkc, p1ps2)
                    elif dg == 6:
                        epi_b(psb, "q0", qT_t[:, 0, pss], True, p1ps2)
                    elif dg == 7:
                        epi_b(psb, "q1", qT_t[:, 1, pss], True, p1ps2)
                        epi_b(psb, "k", kT_t[:, pss], False, p1ps2)
                        vtrans(psb, pv_sb)

            # free the accumulator banks quickly; on the last s-block the
            # v copy goes first so its transposes unblock immediately
            v_sb = None
            if sb == NSB - 1:
                v_sb = epool.tile([P, SB], tdt, tag="vc", bufs=2)
                nc.vector.tensor_copy(v_sb, v_ps)
            copies = []
            for h in range(HG):
                qc = epool.tile([P, SB], f32, tag=f"qc{h}", bufs=2,
                                name=f"qcopy{h}")
                nc.vector.tensor_copy(qc, q_ps[h])
                copies.append(qc)
            kc = epool.tile([P, SB], f32, tag="kc", bufs=2)
            nc.vector.tensor_copy(kc, k_ps)
            if v_sb is None:
                v_sb = epool.tile([P, SB], tdt, tag="vc", bufs=2)
                nc.vector.tensor_copy(v_sb, v_ps)
            prev = (sb, copies, kc, v_sb)

        # last s-block's v transposes only depend on the v copy: emit now,
        # while phase-1 PSUM banks are still open.
        lsb, lcopies, lkc, lv_sb = prev
        vtrans(lsb, lv_sb)
        lss = slice(lsb * SB, (lsb + 1) * SB)
        ph1p.close()
        ph1w.close()

        # ---------------- phase 2: attention ----------------
        e2pool = top.enter_context(tc.tile_pool(name="ep2", bufs=2))
        o3pool = top.enter_context(tc.tile_pool(name="oc", bufs=8))
        expool = top.enter_context(tc.tile_pool(name="exps", bufs=12))
        p2ps = top.enter_context(tc.tile_pool(name="p2ps", bufs=2, space="PSUM"))
        p2sc = top.enter_context(tc.tile_pool(name="p2sc", bufs=1, space="PSUM"))

        def finalize(h, qs_sl, pv_ps, rs_ps):
            # normalization chain, emitted one group late so its PE
            # broadcast matmul never stalls the in-order PE stream.
            pvs = e2pool.tile([P, SB], f32, tag="pvs")
            nc.scalar.copy(pvs, pv_ps)
            rec = e2pool.tile([1, SB], f32, tag="rec")
            nc.vector.reciprocal_approx_fast(out=rec, in_=rs_ps)
            recr = e2pool.tile([1, SB], tdt, tag="recr")
            nc.vector.tensor_copy(recr, rec)
            bc_ps = p2sc.tile([P, SB], f32, tag="ps", bufs=4, name="obc")
            nc.tensor.matmul(
                bc_ps, onesrow_t[:], recr[:], start=True, stop=True,
            )
            bc_sb = e2pool.tile([P, SB], f32, tag="bcc")
            nc.vector.tensor_copy(bc_sb, bc_ps)
            nc.vector.tensor_mul(attnT_t[:, h, qs_sl], pvs, bc_sb)

        def outproj(qb, heads, slot, ccr=None):
            # output written 4 column-chunks per DMA (4KB/partition packets
            # instead of 1KB, and 4x fewer descriptor-generation triggers)
            qsl = slice(qb * SB, (qb + 1) * SB)
            o4 = None
            for cc in (ccr if ccr is not None else range(NCC)):
                o_ps = p2sc.tile([P, SB], f32, tag="ps", bufs=4, name="ops")
                for i, h in enumerate(heads):
                    nc.tensor.matmul(
                        o_ps,
                        wo_t[:, h, cc, :],
                        attnT_t[:, h, qsl],
                        start=(i == 0),
                        stop=(i == len(heads) - 1),
                    )
                if cc % 4 == 0:
                    o4 = o3pool.tile([P, 4, SB], f16, tag="oc", bufs=3)
                if cc % 2 == 0:
                    nc.vector.tensor_copy(o4[:, cc % 4, :], o_ps)
                else:
                    nc.scalar.copy(o4[:, cc % 4, :], o_ps)
                if cc % 4 == 3:
                    nc.gpsimd.dma_start(out_d[cc // 4, slot], o4)

        def attn_group(qb, h, interject=None):
            """Emit one (q-block, head) attention group with 2-deep score
            prefetch. Diagonal-first chunk order with column narrowing; the
            final chunk is always full-width so PSUM accumulation groups
            start/stop over the whole bank. interject: {chunk_idx: fn}
            emits extra (independent) work after that chunk's matmuls."""
            qs_sl = slice(qb * SB, (qb + 1) * SB)
            pv_ps = p2ps.tile([P, SB], f32, tag="pv")
            rs_ps = p2ps.tile([1, SB], f32, tag="rs")
            qt = qT_t[:, h, qs_sl]
            # chunk: (kv_chunk, col0 for sc/exp, pv col0, mask)
            # mask: ("old", j) full-width prefix+triangle | ("tri",) 128-wide
            if qb == 0:
                chunks = [(j, j * P, j * P, ("tri",)) for j in range(3)]
                chunks += [(3, 3 * P, 0, ("old", 3))]
            else:
                chunks = [(4 * qb + j, j * P, j * P, ("tri",)) for j in range(4)]
                chunks += [(c, 0, 0, None) for c in range(4 * qb)]
            n = len(chunks)

            sc_tiles = [None] * n

            def emit_sc(i):
                c, col0, _, _ = chunks[i]
                sc_ps = p2sc.tile([P, SB], f32, tag="ps", bufs=4, name="scps")
                nc.tensor.matmul(
                    sc_ps[:, col0:],
                    kT_t[:, c * P:(c + 1) * P],
                    qt[:, col0:],
                    start=True,
                    stop=True,
                )
                sc_tiles[i] = sc_ps

            def emit_rest(i):
                c, col0, pcol0, mask = chunks[i]
                sc_ps = sc_tiles[i]
                sc_tiles[i] = None
                e_sb = expool.tile([P, SB], tdt, tag="exp")
                nc.scalar.activation(e_sb[:, col0:], sc_ps[:, col0:], Exp)
                if mask is not None:
                    if mask[0] == "old":
                        # prefix+triangle mask also zeroes the (uncomputed)
                        # e_sb columns below col0, enabling full-width pv/rs
                        nc.vector.copy_predicated(
                            e_sb, masks_t[:, mask[1], :], zero_t
                        )
                    else:
                        nc.vector.copy_predicated(
                            e_sb[:, col0:col0 + P], tri_t, zero_t[:, 0:P]
                        )
                st, sp = (i == 0), (i == n - 1)
                nc.tensor.matmul(
                    pv_ps[:, pcol0:], vnat_t[:, c, :], e_sb[:, pcol0:],
                    start=st, stop=sp,
                )
                nc.tensor.matmul(
                    rs_ps[:, pcol0:], onescol_t[:], e_sb[:, pcol0:],
                    start=st, stop=sp,
                )
                if interject and i in interject:
                    interject[i]()

            for i in range(n):
                emit_sc(i)
                if i >= 2:
                    emit_rest(i - 2)
            emit_rest(n - 2)
            emit_rest(n - 1)
            return (qb, h, qs_sl, pv_ps, rs_ps)

        # The last s-block's rope/norm epilogue is interleaved into early
        # phase 2: its outputs are only needed by q-block 3's attention,
        # and the q-block 0/1 groups + finalizes give the PE filler work
        # while the epilogue's DVE/ACT chains drain. All ACT functions
        # (Exp/Ln/Square/Copy) live in one table, so interleaving is free.
        g00 = attn_group(0, 0)
        epi_a(lsb, "q0", lcopies[0], p2sc, ps_full=True, with_sqrt=False)
        g01 = attn_group(0, 1)
        epi_a(lsb, "q1", lcopies[1], p2sc, ps_full=True, with_sqrt=False)

        if DEBUG:
            nc.sync.dma_start(dbg_q, qT_t)
            nc.sync.dma_start(dbg_k, kT_t)
            nc.sync.dma_start(dbg_v, vnat_t)

        # remaining groups, finalize one group late; outproj when both
        # heads of a q-block are done. Last q-block: per-head partials.
        def op_pieces(qb, nch, extra=None):
            # spread the out-projection of q-block qb over the chunks of the
            # next attention group in 4-cc pieces: the PE-only projection
            # matmuls fill the exp-latency of the chunk chain, and the exps
            # (ACT) overlap the projection stream instead of idling.
            d = {}
            for i in range(8):
                lo = i * 4
                key = (i * nch) // 8
                fns = d.setdefault(key, [])
                fns.append(lambda lo=lo: outproj(qb, [0, 1], qb,
                                                 range(lo, lo + 4)))
            if extra:
                for k, fn in extra.items():
                    d.setdefault(k, []).append(fn)
            return {k: (lambda fns=fns: [f() for f in fns])
                    for k, fns in d.items()}

        finalize(g00[1], *g00[2:])
        g10 = attn_group(1, 0, interject={
            6: lambda: epi_a(lsb, "k", lkc, p2sc, ps_full=True,
                             with_sqrt=False)})
        finalize(g01[1], *g01[2:])
        # three consecutive Sqrts: exactly one table switch out of Exp's
        # table and one back, with attention mms as PE filler
        epi_sqrt(lsb, "q0")
        epi_sqrt(lsb, "q1")
        epi_sqrt(lsb, "k")
        epi_b(lsb, "q0", qT_t[:, 0, lss], True, p2sc, ps_full=True)
        epi_b(lsb, "q1", qT_t[:, 1, lss], True, p2sc, ps_full=True)
        epi_b(lsb, "k", kT_t[:, lss], False, p2sc, ps_full=True)
        g11 = attn_group(1, 1, interject=op_pieces(0, 8))
        finalize(g10[1], *g10[2:])
        g20 = attn_group(2, 0)
        finalize(g11[1], *g11[2:])
        g21 = attn_group(2, 1, interject=op_pieces(1, 12))
        finalize(g20[1], *g20[2:])
        g30 = attn_group(3, 0)
        finalize(g21[1], *g21[2:])
        g31 = attn_group(3, 1, interject=op_pieces(
            2, 16, extra={5: lambda: finalize(g30[1], *g30[2:])}))
        # tail: h0's projection is ready (finalized in the interject); its
        # matmuls cover fin(3,1)'s chain so only h1's projection trails.
        outproj(3, [0], 3)
        finalize(g31[1], *g31[2:])
        outproj(3, [1], 4)

        if DEBUG:
            nc.sync.dma_start(dbg_a, attnT_t)

    nc.compile()
    _BUILD_CACHE[key] = nc
    return nc


def _host_prep(x, positions, wq, wk, wv, wo):
    """Returns per-core input maps."""
    npdt = np.float16

    pos_f = positions.astype(np.float32)
    inv_freq = (
        1.0
        / (ROPE_THETA ** (np.arange(0, HEAD_DIM, 2, dtype=np.float32) / HEAD_DIM))
    ).astype(np.float32)
    ang = pos_f[:, None] * inv_freq[None, :]        # [S, 64]
    csT = np.ascontiguousarray(np.cos(ang).T.astype(np.float16))  # [64, S]
    snT = np.ascontiguousarray(np.sin(ang).T.astype(np.float16))  # [64, S]
    attn_scales = (
        np.log(np.floor((pos_f + 1.0) / FLOOR_SCALE) + 1.0) * ATTN_SCALE + 1.0
    )
    qscale = (attn_scales / np.sqrt(np.float32(HEAD_DIM))).astype(np.float32)[None, :]

    # rotate-half permutation of q/k feature dims (per head), folded into
    # the projection weight columns: permuted feature j<64 <- 2j, j>=64 <- 2(j-64)+1
    perm = np.concatenate([np.arange(0, HEAD_DIM, 2), np.arange(1, HEAD_DIM, 2)])
    wq_p = wq.reshape(D, N_HEADS, HEAD_DIM)[:, :, perm].reshape(D, N_HEADS * HEAD_DIM)
    wk_p = wk[:, perm]

    def tile_x(xT):
        # [D, S] -> [sb, dg, p, c, s]
        return np.ascontiguousarray(
            xT.reshape(8, 4, P, NSB, SB).transpose(3, 0, 2, 1, 4)
        )

    def tile_w(w):
        # [D, m] -> [dg, p, c, m]
        m = w.shape[1]
        return np.ascontiguousarray(
            w.reshape(8, 4, P, m).transpose(0, 2, 1, 3)
        )

    def tile_wo(wg):
        # [256, D] -> [p, hh, cc, q]
        return np.ascontiguousarray(
            wg.reshape(HG, P, NCC, P).transpose(1, 0, 2, 3)
        )

    in_maps = []
    for core in range(8):
        b, g = core // 2, core % 2
        xT = np.ascontiguousarray(x[b].T).astype(npdt, copy=False)
        in_maps.append(
            {
                "xT": tile_x(xT),
                "wq_g": tile_w(
                    wq_p[:, g * HG * HEAD_DIM:(g + 1) * HG * HEAD_DIM].astype(npdt)
                ),
                "wk": tile_w(wk_p.astype(npdt)),
                "wv": tile_w(wv.astype(npdt)),
                "wo_g": tile_wo(
                    wo[g * HG * HEAD_DIM:(g + 1) * HG * HEAD_DIM, :].astype(npdt)
                ),
                "csT": csT,
                "snT": snT,
                "qscale": qscale,
            }
        )
    return in_maps


def kernel(x, positions, wq, wk, wv, wo, _trace=False, _trace_kwargs=None):
    x = np.asarray(x, np.float32)
    positions = np.asarray(positions)
    wq = np.asarray(wq, np.float32)
    wk = np.asarray(wk, np.float32)
    wv = np.asarray(wv, np.float32)
    wo = np.asarray(wo, np.float32)

    nc = build_bass()
    in_maps = _host_prep(x, positions, wq, wk, wv, wo)
    res = bass_utils.run_bass_kernel_spmd(
        nc, in_maps, core_ids=list(range(8)), trace=_trace,
        **(_trace_kwargs or {}),
    )
    kernel.last_results = res

    out = np.empty((B, S, D), np.float32)
    for b in range(B):
        pa = res.results[2 * b]["outT"].astype(np.float32)
        pb = res.results[2 * b + 1]["outT"].astype(np.float32)
        # [cc4, slot, p, c, s]
        pa[:, 3] += pa[:, 4]
        pb[:, 3] += pb[:, 4]
        comb = pa[:, :NSB] + pb[:, :NSB]          # [cc4, qb, p, c, s]
        full = comb.transpose(0, 3, 2, 1, 4).reshape(D, S)
        out[b] = full.T
    return out
